# revision 1
# baseline (speedup 1.0000x reference)
"""DSAttention (de-stationary causal attention) Trainium2 Bass kernel.

Problem: B=4, L=S=2048, H=8, E=D=64, f32.
  scores = (Q @ K^T) * tau[b] + delta[b, j]
  A = softmax(scale * scores + causal mask), scale = 1/sqrt(E)
  out = A @ V

Sharding: B*H = 32 independent (b,h) attention heads -> 4 per core on 8 cores.

Device algorithm (per (b,h) pair), S^T formulation so the softmax reduction
falls out of the PE:
  - Host pre-transposes Q,K to [E, L] ("e on partitions") and folds the
    de-stationary terms into the matmul:
      qk[0:64, i]      = 0.125 * tau[b] * Q[i, :]^T ; qk[64, i] = 1        (qt)
      qk[0:64, 2048+j] = K[j, :]^T ; qk[64, 2048+j] = 0.125*delta[b,j]     (kt)
    => S'^T[j, i] = 0.125 * (tau * (Q K^T) + delta)[j, i]  (the exact logits)
  - Causal mask: additive 0/-1e30 [128,128] block on the PSUM diagonal tile
    (DVE), then one wide exp (ACT, f32 PSUM -> f32r SBUF).
  - PV: V is augmented with a ones column (host) so a single accumulating
    matmul yields the numerator O'^T (rows 0..63) AND the softmax denominator
    (row 64) in PSUM.
  - Unnormalized [65, 2048] result returns to HBM; the host divides by row 64
    and transposes into the (B, L, H, D) output.
  - All matmuls run in float32r (full-rate fp32 PE mode).
"""

import numpy as np

B, L, SEQ, H, E, D = 4, 2048, 2048, 8, 64, 64
N_CORES = 8
PAIRS = (B * H) // N_CORES  # 4 (b,h) pairs per core
SCALE = 1.0 / float(np.sqrt(E))  # 0.125
JC = 128               # j-chunk (key rows per tile, PSUM partition dim)
IH = 1024              # i-half width (PSUM free dim budget)
N_CHUNKS = SEQ // JC   # 16
NEG = -1.0e30

_CACHED = {}


def _build_bass(reps=1):
    key = ("nc", reps)
    if key in _CACHED:
        return _CACHED[key]
    import concourse.mybir as mybir
    import concourse.tile as tile
    from concourse import bacc

    f32 = mybir.dt.float32
    f32r = mybir.dt.float32r
    EXP = mybir.ActivationFunctionType.Exp

    nc = bacc.Bacc("TRN2", target_bir_lowering=False, debug=False)

    qk = nc.dram_tensor("qk", [PAIRS, E + 1, 2 * L], f32r,
                        kind="ExternalInput").ap()
    vp = nc.dram_tensor("vp", [PAIRS, JC, N_CHUNKS * (D + 1)], f32r,
                        kind="ExternalInput").ap()
    trim = nc.dram_tensor("trim", [JC, JC], f32r, kind="ExternalInput").ap()
    o = nc.dram_tensor("o", [PAIRS, D + 1, L], f32, kind="ExternalOutput").ap()

    with tile.TileContext(nc) as tc:
        with (
            tc.tile_pool(name="const", bufs=1) as const_pool,
            tc.tile_pool(name="qkp", bufs=2) as qk_pool,
            tc.tile_pool(name="vpool", bufs=2) as v_pool,
            tc.tile_pool(name="et", bufs=3) as et_pool,
            tc.tile_pool(name="ot", bufs=2) as ot_pool,
            tc.tile_pool(name="ps", bufs=2, space="PSUM") as ps_pool,
            tc.tile_pool(name="po", bufs=2, space="PSUM") as po_pool,
        ):
            trim_t = const_pool.tile([JC, JC], f32r, name="trim_t")
            nc.sync.dma_start(out=trim_t[:], in_=trim[:])

            for rep in range(reps):
              for p in range(PAIRS):
                  qk_t = qk_pool.tile([E + 1, 2 * L], f32r, tag="qk",
                                      name=f"qk{rep}_{p}")
                  vp_t = v_pool.tile([JC, N_CHUNKS * (D + 1)], f32r, tag="vp",
                                     name=f"vp{rep}_{p}")
                  nc.sync.dma_start(out=qk_t[:], in_=qk[p])
                  nc.sync.dma_start(out=vp_t[:], in_=vp[p])

                  for half in range(L // IH):
                      i_lo = half * IH
                      po_t = po_pool.tile([D + 1, IH], f32, tag="po",
                                          name=f"po{rep}_{p}_{half}")
                      nchunks = (i_lo + IH) // JC
                      # last chunk touching each 512-col PSUM bank (the sim's
                      # accumulation-group stop flag is bank-granular)
                      last_c = {}
                      for c in range(nchunks):
                          xc = max(0, JC * c - i_lo)
                          for b0 in range(0, IH, 512):
                              if max(xc, b0) < b0 + 512:
                                  last_c[b0] = c
                      for c in range(nchunks):
                          j0 = JC * c
                          a0 = max(i_lo, j0)     # first valid (causal) i col
                          x = a0 - i_lo          # offset within the i-half
                          w = IH - x             # valid width
                          ps_t = ps_pool.tile([JC, IH], f32, tag="ps",
                                              name=f"ps{rep}_{p}_{half}_{c}")
                          # S'^T = kt_chunk.T @ qt  into per-bank slices
                          for b0 in range(0, IH, 512):
                              lo = max(x, b0)
                              b1 = b0 + 512
                              if lo < b1:
                                  nc.tensor.matmul(
                                      ps_t[:, lo:b1],
                                      qk_t[:, L + j0:L + j0 + JC],
                                      qk_t[:, i_lo + lo:i_lo + b1],
                                      start=True, stop=True,
                                  )
                          et_t = et_pool.tile([JC, IH], f32r, tag="et",
                                              name=f"et{rep}_{p}_{half}_{c}")
                          nc.scalar.activation(et_t[:, 0:w], ps_t[:, x:IH], EXP)
                          if j0 >= i_lo:
                              # diagonal block: keep j <= i
                              nc.vector.tensor_mul(
                                  et_t[:, 0:JC], et_t[:, 0:JC], trim_t[:])
                          # O'^T += vp_chunk.T @ exp(S'^T)
                          first = c == 0
                          for b0 in range(0, IH, 512):
                              lo = max(x, b0)
                              b1 = b0 + 512
                              if lo < b1:
                                  nc.tensor.matmul(
                                      po_t[:, lo:b1],
                                      vp_t[:, c * (D + 1):(c + 1) * (D + 1)],
                                      et_t[:, lo - x:b1 - x],
                                      start=first,
                                      stop=(c == last_c[b0]),
                                  )
                      ot_t = ot_pool.tile([D + 1, IH], f32, tag="ot",
                                          name=f"ot{rep}_{p}_{half}")
                      nc.vector.tensor_copy(ot_t[:], po_t[:])
                      nc.sync.dma_start(out=o[p][:, i_lo:i_lo + IH], in_=ot_t[:])

    nc.compile()
    _CACHED[key] = nc
    return nc


def _prep_core_inputs(queries, keys, values, tau, delta, core):
    qk = np.empty((PAIRS, E + 1, 2 * L), dtype=np.float32)
    vp = np.empty((PAIRS, JC, N_CHUNKS * (D + 1)), dtype=np.float32)
    for p in range(PAIRS):
        g = core * PAIRS + p
        b, h = g // H, g % H
        qk[p, :E, :L] = (SCALE * tau[b, 0]) * queries[b, :, h, :].T
        qk[p, E, :L] = 1.0
        qk[p, :E, L:] = keys[b, :, h, :].T
        qk[p, E, L:] = SCALE * delta[b, :]
        v = values[b, :, h, :].reshape(N_CHUNKS, JC, D)
        vch = vp[p].reshape(JC, N_CHUNKS, D + 1)
        vch[:, :, :D] = v.transpose(1, 0, 2)
        vch[:, :, D] = 1.0
    trim = np.triu(np.ones((JC, JC), dtype=np.float32))
    return {"qk": qk, "vp": vp, "trim": trim}


def _run(queries, keys, values, tau, delta, trace=False, trace_kwargs=None):
    from concourse.bass_utils import run_bass_kernel_spmd

    queries = np.asarray(queries, dtype=np.float32)
    keys = np.asarray(keys, dtype=np.float32)
    values = np.asarray(values, dtype=np.float32)
    tau = np.asarray(tau, dtype=np.float32)
    delta = np.asarray(delta, dtype=np.float32)

    nc = _build_bass()
    in_maps = [
        _prep_core_inputs(queries, keys, values, tau, delta, core)
        for core in range(N_CORES)
    ]
    res = run_bass_kernel_spmd(
        nc, in_maps, list(range(N_CORES)), trace=trace,
        **(trace_kwargs or {}),
    )

    out = np.empty((B, L, H, D), dtype=np.float32)
    for core in range(N_CORES):
        o = res.results[core]["o"]  # [PAIRS, 65, L]
        for p in range(PAIRS):
            g = core * PAIRS + p
            b, h = g // H, g % H
            out[b, :, h, :] = (o[p, :D, :] / o[p, D:D + 1, :]).T
    return out, res


def kernel(queries, keys, values, tau, delta):
    out, _ = _run(queries, keys, values, tau, delta)
    return out

